# revision 22
# baseline (speedup 1.0000x reference)
"""Trainium2 Bass kernel for nn_C3S_RegularLoss.

reference:
    xr = x.reshape(B, P, D); xn = xr / ||xr||_2(axis=-1)
    s = mean_b(xn)                     # (P, D)
    corr = s @ s.T                     # (P, P)
    loss = (sum(corr) - 3*trace(corr) + 2P) / 2 * gamma

Reformulated without the corr matrix:
    sum(corr)   = || sum_p s_p ||^2
    trace(corr) = sum_p || s_p ||^2
so with S = sum_b xn (sum, not mean):
    loss = ((||sum_p S_p||^2 - 3*sum(S^2)) / B^2 + 2P) / 2 * gamma

Sharding: data-parallel over the batch dim, 8 cores x 1024 rows.
Each core computes S_partial = sum_b r_b * x_b per part via PE matmuls
(r = 1/||x_part|| as the stationary operand), AllReduce of the (4,2048)
sums, then a tiny replicated tail computes the scalar loss.
"""

import os
import sys

sys.path.insert(0, "/opt/trn_rl_repo")
os.environ.setdefault("MYCRO_LOCAL_CACHE", "1")

import numpy as np

B, F = 8192, 8192
NPARTS = 4
D = F // NPARTS                 # 2048
NCORES = 8
B_CORE = B // NCORES            # 1024
TILE_P = 128
NTILES = B_CORE // TILE_P       # 8
MM_N = 512                      # moving free dim per matmul
NCHUNK = D // MM_N              # 4

_cache = {}


def _build(ncores=NCORES, collective=True):
    import concourse.bass as bass  # noqa: F401
    import concourse.mybir as mybir
    from concourse import bacc, tile
    from concourse.tile import add_dep_helper

    f32 = mybir.dt.float32
    bf16 = mybir.dt.bfloat16
    Act = mybir.ActivationFunctionType
    Alu = mybir.AluOpType

    nc = bacc.Bacc("TRN2", num_devices=ncores, debug=False)
    x_t = nc.dram_tensor("x", [B_CORE, F], f32, kind="ExternalInput")
    g_t = nc.dram_tensor("gamma", [1, 1], f32, kind="ExternalInput")
    out_t = nc.dram_tensor("out", [1, 1], f32, kind="ExternalOutput")

    with tile.TileContext(nc) as tc:
        with tc.tile_pool(name="xp", bufs=7) as xp, \
             tc.tile_pool(name="scratch", bufs=2) as scp, \
             tc.tile_pool(name="small", bufs=3) as stp, \
             tc.tile_pool(name="tail", bufs=1) as tlp, \
             tc.tile_pool(name="ps", bufs=1, space="PSUM") as psp, \
             tc.tile_pool(name="dram", bufs=1, space="DRAM") as dram:

            # PSUM accumulators: part p lives at psum partition 32*p
            # (PE col tile_position constraint). Two 4-bank accumulators:
            # tiles 0-3 -> S_a (AllReduce'd mid-kernel, hidden under the
            # DMA stream and absorbing rank skew), tiles 4-7 -> S_b
            # (small aligned AllReduce at the end).
            S_a = psp.tile([TILE_P, D], f32, tag="accA")
            S_b = psp.tile([TILE_P, D], f32, tag="accB")
            cc_in_a = dram.tile([NPARTS, D], f32)
            cc_out_a = dram.tile([NPARTS, D], f32)
            cc_in_b = dram.tile([NPARTS, D], f32)
            cc_out_b = dram.tile([NPARTS, D], f32)
            # AR1 covers only tile 0 so it triggers as early as possible:
            # its completion (which includes waiting for the most-skewed
            # rank) then lands well before AR2 is ready, so AR2 never
            # queues behind it on the collective stream.
            HALF = 1

            prev_sqrt = None
            prev_cast = None
            for i in range(NTILES):
                last = i == NTILES - 1
                # SWDGE DMA casts fp32 -> bf16 in-flight (free; PE wants
                # bf16 and the loss has ~1e3x precision headroom).
                # Last tile: split per part so its (fully exposed)
                # normalize chain starts at the first part boundary.
                xt = xp.tile([TILE_P, F], bf16, tag="xt")
                rows = x_t[i * TILE_P:(i + 1) * TILE_P, :]
                if last:
                    for p in range(NPARTS):
                        nc.gpsimd.dma_start(xt[:, p * D:(p + 1) * D],
                                            rows[:, p * D:(p + 1) * D])
                else:
                    nc.gpsimd.dma_start(xt[:], rows)

                # sum-of-squares per part, all on ACT (square + free
                # accumulator). Keeping the big elementwise ops OFF the
                # vector engine matters: DVE SBUF reads lock GpSimd out
                # of the port it uses for SWDGE descriptor rings, which
                # stalls the x-tile DMA stream.
                # Last tile: give one part to DVE to shorten the final
                # (non-overlapped) r-chain; DMAs are done by then.
                ss = stp.tile([TILE_P, NPARTS], f32, tag="ss")
                sqa = scp.tile([TILE_P, D], bf16, tag="sqa")
                norm = stp.tile([TILE_P, NPARTS], f32, tag="norm")
                r = stp.tile([TILE_P, NPARTS], f32, tag="r")
                r_bf = stp.tile([TILE_P, NPARTS], bf16, tag="r_bf")
                S_ps = S_a if i < HALF else S_b

                def mms_for_part(p, rbf_ap):
                    for j in range(NCHUNK):
                        nc.tensor.matmul(
                            S_ps[32 * p:32 * p + 1, j * MM_N:(j + 1) * MM_N],
                            lhsT=rbf_ap,
                            rhs=xt[:, p * D + j * MM_N:p * D + (j + 1) * MM_N],
                            start=(i == 0 or i == HALF),
                            stop=(i == HALF - 1 or i == NTILES - 1),
                            tile_position=(0, 32 * p))

                if not last:
                    for p in range(NPARTS):
                        a = nc.scalar.activation(
                            sqa[:], xt[:, p * D:(p + 1) * D], Act.Square,
                            accum_out=ss[:, p:p + 1])
                        if p == 0 and prev_sqrt is not None:
                            # pin ACT order: sqrt(i-1) must precede
                            # squares(i), else the scheduler makes r(i-1)
                            # wait on DMA(i)
                            add_dep_helper(
                                a.ins, prev_sqrt.ins, sync=False,
                                reason="sqrt(i-1) before squares(i)")
                    prev_sqrt = nc.scalar.sqrt(norm[:], ss[:])
                    nc.vector.reciprocal(r[:], norm[:])
                    prev_cast = nc.vector.tensor_copy(r_bf[:], r[:])
                    for p in range(NPARTS):
                        mms_for_part(p, r_bf[:, p:p + 1])
                else:
                    # per-part chain: square -> sqrt -> recip -> cast ->
                    # matmuls, so part p's work starts as soon as its
                    # quarter of the final DMA lands
                    pa = None
                    for p in range(NPARTS):
                        a = nc.scalar.activation(
                            sqa[:], xt[:, p * D:(p + 1) * D], Act.Square,
                            accum_out=ss[:, p:p + 1])
                        if p == 0 and prev_sqrt is not None:
                            add_dep_helper(a.ins, prev_sqrt.ins, sync=False,
                                           reason="sqrt(i-1) first")
                        if pa is not None:
                            add_dep_helper(a.ins, pa.ins, sync=False,
                                           reason="ACT part order")
                        pa = nc.scalar.sqrt(norm[:, p:p + 1], ss[:, p:p + 1])
                        nc.vector.reciprocal(r[:, p:p + 1], norm[:, p:p + 1])
                        nc.vector.tensor_copy(r_bf[:, p:p + 1], r[:, p:p + 1])
                        mms_for_part(p, r_bf[:, p:p + 1])

                if i == HALF - 1:
                    # first-half partial sums: ship out + AllReduce now,
                    # overlapped with the second half of the DMA stream
                    s_sba = tlp.tile([TILE_P, D], f32, tag="s_sba")
                    nc.vector.tensor_copy(s_sba[:], S_a[:])
                    for p in range(NPARTS):
                        nc.sync.dma_start(cc_in_a[p:p + 1, :],
                                          s_sba[32 * p:32 * p + 1, :])
                    if collective:
                        nc.gpsimd.collective_compute(
                            "AllReduce", Alu.add,
                            replica_groups=[list(range(ncores))],
                            ins=[cc_in_a.opt()], outs=[cc_out_a.opt()])
                    else:
                        nc.sync.dma_start(cc_out_a[:], cc_in_a[:])

            # ---- second-half partial sums: AllReduce over 8 cores ----
            # one full-width PSUM->SBUF copy (rows besides 0/32/64/96 are
            # junk but harmless) instead of 4 serial row copies
            s_sb = tlp.tile([TILE_P, D], f32, tag="s_sb")
            nc.scalar.copy(s_sb[:, :D // 2], S_b[:, :D // 2])
            nc.vector.tensor_copy(s_sb[:, D // 2:], S_b[:, D // 2:])

            for p in range(NPARTS):
                eng = nc.sync if p % 2 == 0 else nc.scalar
                eng.dma_start(cc_in_b[p:p + 1, :],
                              s_sb[32 * p:32 * p + 1, :])
            ar2 = None
            if collective:
                ar2 = nc.gpsimd.collective_compute(
                    "AllReduce", Alu.add,
                    replica_groups=[list(range(ncores))],
                    ins=[cc_in_b.opt()], outs=[cc_out_b.opt()])
            else:
                nc.sync.dma_start(cc_out_b[:], cc_in_b[:])

            # reload both summed halves as bf16 (cast in DMA) and add
            sfa = tlp.tile([NPARTS, D], bf16, tag="sfa")
            ld_a = nc.gpsimd.dma_start(sfa[:], cc_out_a[:])
            if ar2 is not None:
                # keep gpsimd free to fire the AR2 doorbell before it
                # blocks on AR1's output
                add_dep_helper(ld_a.ins, ar2.ins, sync=False,
                               reason="AR2 doorbell before sfa load")
            sfb = tlp.tile([NPARTS, D], bf16, tag="sfb")
            nc.gpsimd.dma_start(sfb[:], cc_out_b[:])
            sfull = tlp.tile([NPARTS, D], bf16, tag="sfull")
            nc.vector.tensor_add(sfull[:], sfa[:], sfb[:])

            # ---- replicated tail: loss scalar ----
            ones4 = tlp.tile([NPARTS, 1], bf16, tag="ones4")
            nc.vector.memset(ones4[:], 1.0)
            ones4f = tlp.tile([NPARTS, 1], f32, tag="ones4f")
            nc.vector.memset(ones4f[:], 1.0)

            # B2 = sum(S^2) over everything — on DVE (mult + reduce) so
            # it runs in parallel with ACT's ||t||^2 square below
            sq_tail = tlp.tile([NPARTS, D], bf16, tag="sq_tail")
            ssum = tlp.tile([NPARTS, 1], f32, tag="ssum")
            nc.vector.tensor_mul(sq_tail[:], sfull[:], sfull[:])
            nc.vector.reduce_sum(ssum[:], sq_tail[:],
                                 axis=mybir.AxisListType.X)
            b2_ps = psp.tile([1, 1], f32, tag="accB")
            nc.tensor.matmul(b2_ps[:], lhsT=ones4f[:], rhs=ssum[:],
                             start=True, stop=True)

            # t = sum_p S_p  (K=4 matmul with ones), then A = ||t||^2
            # (tag "acc": reuse the S accumulator's PSUM banks — S is dead
            # once copied to s_sb)
            t_ps = psp.tile([1, D], f32, tag="accA")
            for j in range(NCHUNK):
                nc.tensor.matmul(
                    t_ps[0:1, j * MM_N:(j + 1) * MM_N],
                    lhsT=ones4[:],
                    rhs=sfull[:, j * MM_N:(j + 1) * MM_N],
                    start=True, stop=True)
            t_sq = tlp.tile([1, D], f32, tag="t_sq")
            a_sb = tlp.tile([1, 1], f32, tag="a_sb")
            nc.scalar.activation(t_sq[:], t_ps[:], Act.Square,
                                 accum_out=a_sb[:])

            # loss = ((A - 3*B2) / B^2 + 2P) / 2 * gamma
            g_sb = tlp.tile([1, 1], f32, tag="g_sb")
            nc.sync.dma_start(g_sb[:], g_t[:])
            tmp = tlp.tile([1, 1], f32, tag="tmp")
            nc.vector.tensor_scalar(
                out=tmp[:], in0=b2_ps[:], scalar1=-3.0, scalar2=None,
                op0=Alu.mult)
            tt = tlp.tile([1, 1], f32, tag="tt")
            nc.vector.tensor_add(tt[:], tmp[:], a_sb[:])
            l0 = tlp.tile([1, 1], f32, tag="l0")
            nc.vector.tensor_scalar(
                out=l0[:], in0=tt[:],
                scalar1=1.0 / (2.0 * float(B) * float(B)),
                scalar2=float(NPARTS),
                op0=Alu.mult, op1=Alu.add)
            loss = tlp.tile([1, 1], f32, tag="loss")
            nc.vector.tensor_mul(loss[:], l0[:], g_sb[:])
            nc.sync.dma_start(out_t[:], loss[:])

    nc.compile()
    return nc


def _get_nc():
    if "nc" not in _cache:
        _cache["nc"] = _build()
    return _cache["nc"]


def kernel(x, gamma, **run_kwargs):
    from concourse import bass_utils

    x = np.ascontiguousarray(np.asarray(x, dtype=np.float32))
    gamma = np.asarray(gamma, dtype=np.float32).reshape(1, 1)
    assert x.shape == (B, F), x.shape

    nc = _get_nc()
    in_maps = [
        {"x": x[c * B_CORE:(c + 1) * B_CORE], "gamma": gamma}
        for c in range(NCORES)
    ]
    res = bass_utils.run_bass_kernel_spmd(
        nc, in_maps, core_ids=list(range(NCORES)), **run_kwargs)
    out = np.asarray(res.results[0]["out"], dtype=np.float32).reshape(1)
    if run_kwargs.get("trace"):
        _cache["last_results"] = res
    return out
